# revision 1
# baseline (speedup 1.0000x reference)
"""Code2VecEncoder Trainium2 kernel.

Strategy (8 NeuronCores):
  - cores 0-3: extract #1, batch rows [128c, 128(c+1))
  - cores 4-7: extract #2, same batch split
  Each core handles 128 batch rows x 200 paths of one extract end-to-end;
  no cross-core communication. Host concatenates per-core [128, 384]
  outputs into the two [512, 384] code vectors.

Per-core pipeline (l-major: token i = l*128 + b):
  1. phase-1 dma_gather: bucketed (int16-range) embedding-row gathers,
     HBM -> SBUF staging, bf16 rows (3 streams: word[xs], path, word[xt]).
  2. phase-2 dma_gather(transpose=True, SBUF source): permutation indices
     (token order) -> ctxT [f, tok] bf16. This both un-sorts the bucketed
     staging AND transposes rows into matmul ("f on partitions") layout.
  3. PE: pre[b, d] = sum_s ctxT_s.T @ WT_s  (bf16, fp32 PSUM accum)
  4. ACT: h = tanh(pre) -> bf16
  5. DVE: s[b] = sum_d h*attn  (fused tensor_tensor_reduce)
  6. ACT: e = exp(s)   (no max subtraction needed: |s| ~ 1)
  7. DVE: diag(e) = identity * e ; PE: cv_psum += diag(e) @ h  (200 accum)
  8. Z = sum_l e ; cv = cv_psum / Z ; DMA out.
"""

import numpy as np
import ml_dtypes

import concourse.bacc as bacc
import concourse.mybir as mybir
import concourse.tile as tile
from concourse import bass
from concourse.bass_utils import run_bass_kernel_spmd

BF16 = ml_dtypes.bfloat16

B, L = 512, 200
WORD_V, PATH_V = 100000, 200000
E, D = 128, 384
NCORES = 8
BC = 128                      # batch rows per core
BUCKET = 32768

CL = 100                      # l-values per staging chunk
NCHUNK = L // CL              # 2
CHUNK_TOK = CL * BC           # 12800 tokens per chunk per stream
SUB_TOK = 3200                # phase-2 subchunk (25 l)
NSUB = CHUNK_TOK // SUB_TOK   # 4
SUB_L = SUB_TOK // BC         # 25
NL_T = 3                      # l per PSUM-pre tile / tanh group

WORD_CAPS = [4864, 4864, 4864, 640]          # padded bucket slot caps
PATH_CAPS = [2816] * 6 + [640]
WORD_SLOTS = sum(WORD_CAPS)                   # 15232
PATH_SLOTS = sum(PATH_CAPS)                   # 17536
STREAM_CAPS = [WORD_CAPS, PATH_CAPS, WORD_CAPS]
STREAM_SLOTS = [WORD_SLOTS, PATH_SLOTS, WORD_SLOTS]
IDX1_CHUNK_COLS = sum(STREAM_SLOTS) // 16     # 3000
IDX2_CHUNK_COLS = 3 * (CHUNK_TOK // 16)       # 2400

_nc_cache = {}


def _l_groups():
    gs = []
    l0 = 0
    while l0 < SUB_L:
        g = min(NL_T, SUB_L - l0)
        gs.append((l0, g))
        l0 += g
    return gs


def build_program():
    if "nc" in _nc_cache:
        return _nc_cache["nc"]
    nc = bacc.Bacc(
        "TRN2",
        target_bir_lowering=False,
        debug=False,
        enable_asserts=False,
        num_devices=NCORES,
    )
    dt = mybir.dt
    tab_w = nc.dram_tensor("tab_w", [WORD_V, E], dt.bfloat16, kind="ExternalInput").ap()
    tab_p = nc.dram_tensor("tab_p", [PATH_V, E], dt.bfloat16, kind="ExternalInput").ap()
    idx1 = nc.dram_tensor("idx1", [128, NCHUNK * IDX1_CHUNK_COLS], dt.int16, kind="ExternalInput").ap()
    idx2 = nc.dram_tensor("idx2", [128, NCHUNK * IDX2_CHUNK_COLS], dt.int16, kind="ExternalInput").ap()
    wt = nc.dram_tensor("wt", [128, 3, D], dt.bfloat16, kind="ExternalInput").ap()
    attn_rep = nc.dram_tensor("attn_rep", [128, D], dt.bfloat16, kind="ExternalInput").ap()
    ident = nc.dram_tensor("ident", [128, 128], dt.bfloat16, kind="ExternalInput").ap()
    out = nc.dram_tensor("out", [BC, D], dt.float32, kind="ExternalOutput").ap()

    tabs = [tab_w, tab_p, tab_w]

    with tile.TileContext(nc) as tc:
        with (
            tc.tile_pool(name="const", bufs=1) as constp,
            tc.tile_pool(name="stag", bufs=1) as stagp,
            tc.tile_pool(name="ctxT", bufs=2) as ctxp,
            tc.tile_pool(name="h", bufs=4) as hp,
            tc.tile_pool(name="small", bufs=3) as smallp,
            tc.tile_pool(name="pre", bufs=2, space="PSUM") as prep,
            tc.tile_pool(name="cvp", bufs=1, space="PSUM") as cvp,
        ):
            wt_sb = constp.tile([128, 3, D], dt.bfloat16)
            nc.sync.dma_start(out=wt_sb[:], in_=wt[:])
            attn_sb = constp.tile([128, D], dt.bfloat16)
            nc.sync.dma_start(out=attn_sb[:], in_=attn_rep[:])
            id_sb = constp.tile([128, 128], dt.bfloat16)
            nc.sync.dma_start(out=id_sb[:], in_=ident[:])
            idx1_sb = constp.tile([128, NCHUNK * IDX1_CHUNK_COLS], dt.int16)
            nc.sync.dma_start(out=idx1_sb[:], in_=idx1[:])
            idx2_sb = constp.tile([128, NCHUNK * IDX2_CHUNK_COLS], dt.int16)
            nc.sync.dma_start(out=idx2_sb[:], in_=idx2[:])

            s_all = constp.tile([128, L], dt.float32)
            e_all = constp.tile([128, L], dt.float32)
            junk = constp.tile([128, D], dt.bfloat16)
            zcol = constp.tile([128, 1], dt.float32)
            rz = constp.tile([128, 1], dt.float32)
            cv_sb = constp.tile([128, D], dt.float32)

            cv_ps = cvp.tile([128, 512], dt.float32)

            stream_tags = ["stg_s", "stg_p", "stg_t"]

            for c in range(NCHUNK):
                stags = [
                    stagp.tile(
                        [128, STREAM_SLOTS[s] // 128, E], dt.bfloat16,
                        tag=stream_tags[s], name=f"{stream_tags[s]}_{c}",
                    )
                    for s in range(3)
                ]
                # ---- phase 1: bucketed HBM gathers into staging ----
                col = c * IDX1_CHUNK_COLS
                for s in range(3):
                    tab = tabs[s]
                    vocab = tab.shape[0]
                    blk0 = 0
                    for k, cap in enumerate(STREAM_CAPS[s]):
                        r0 = k * BUCKET
                        r1 = min(vocab, r0 + BUCKET)
                        nc.gpsimd.dma_gather(
                            out_ap=stags[s][:, blk0:blk0 + cap // 128, :],
                            in_ap=tab[r0:r1, :],
                            idxs_ap=idx1_sb[:, col:col + cap // 16],
                            num_idxs=cap,
                            num_idxs_reg=cap,
                            elem_size=E,
                            single_packet=False,
                        )
                        col += cap // 16
                        blk0 += cap // 128

                # ---- phase 2 + compute, per 3200-token subchunk ----
                for sub in range(NSUB):
                    ctxT = ctxp.tile([128, 3, SUB_TOK], dt.bfloat16)
                    for s in range(3):
                        i2col = c * IDX2_CHUNK_COLS + s * (CHUNK_TOK // 16) + sub * (SUB_TOK // 16)
                        nc.gpsimd.dma_gather(
                            out_ap=ctxT[:, s:s + 1, :],
                            in_ap=stags[s][:],
                            idxs_ap=idx2_sb[:, i2col:i2col + SUB_TOK // 16],
                            num_idxs=SUB_TOK,
                            num_idxs_reg=SUB_TOK,
                            elem_size=E,
                            transpose=True,
                            sbuf_tokens_per_rank=128,
                            sbuf_free_dim_per_rank=E * 2,
                            sbuf_free_dim_pad_per_rank=0,
                            sbuf_byte_offset=0,
                            single_packet=False,
                        )

                    for (lg0, g) in _l_groups():
                        pre = prep.tile([128, NL_T, 512], dt.float32, tag="pre")
                        h = hp.tile([128, NL_T, D], dt.bfloat16, tag="h")
                        base_l = c * CL + sub * SUB_L + lg0
                        for j in range(g):
                            t0 = (lg0 + j) * BC
                            for s in range(3):
                                nc.tensor.matmul(
                                    pre[:, j, :D],
                                    lhsT=ctxT[:, s, t0:t0 + BC],
                                    rhs=wt_sb[:, s, :],
                                    start=(s == 0),
                                    stop=(s == 2),
                                    skip_group_check=True,
                                )
                        nc.scalar.activation(
                            h[:, :g, :], pre[:, :g, :D],
                            mybir.ActivationFunctionType.Tanh,
                        )
                        for j in range(g):
                            nc.vector.scalar_tensor_tensor(
                                out=junk[:],
                                in0=h[:, j, :],
                                scalar=1.0,
                                in1=attn_sb[:],
                                op0=mybir.AluOpType.bypass,
                                op1=mybir.AluOpType.mult,
                                accum_out=s_all[:, base_l + j:base_l + j + 1],
                            )
                        nc.scalar.activation(
                            e_all[:, base_l:base_l + g],
                            s_all[:, base_l:base_l + g],
                            mybir.ActivationFunctionType.Exp,
                        )
                        for j in range(g):
                            lglob = base_l + j
                            diag = smallp.tile([128, 128], dt.bfloat16, tag="diag")
                            nc.vector.tensor_scalar_mul(
                                diag[:], id_sb[:], e_all[:, lglob:lglob + 1]
                            )
                            nc.tensor.matmul(
                                cv_ps[:, :D],
                                lhsT=diag[:],
                                rhs=h[:, j, :],
                                start=(lglob == 0),
                                stop=(lglob == L - 1),
                                skip_group_check=True,
                            )

            nc.vector.tensor_reduce(
                out=zcol[:], in_=e_all[:], axis=mybir.AxisListType.X,
                op=mybir.AluOpType.add,
            )
            nc.vector.reciprocal(rz[:], zcol[:])
            nc.scalar.activation(
                cv_sb[:], cv_ps[:, :D],
                mybir.ActivationFunctionType.Copy,
                scale=rz[:, :1],
            )
            nc.sync.dma_start(out=out[:], in_=cv_sb[:])

    nc.compile()
    _nc_cache["nc"] = nc
    return nc


def _wrap16(vals, ncols):
    """int16 values j -> partition j%16, col j//16; replicated to 128 parts."""
    m = np.zeros((16, ncols), dtype=np.int16)
    j = np.arange(len(vals))
    m[j % 16, j // 16] = vals
    return np.tile(m, (8, 1))


def _prep_indices(xs, path, xt):
    """Build idx1/idx2 arrays for one core.

    xs/path/xt: int arrays [128, 200] (this core's shard).
    Token order within chunk c: i = l_local*128 + b.
    """
    idx1 = np.zeros((128, NCHUNK * IDX1_CHUNK_COLS), dtype=np.int16)
    idx2 = np.zeros((128, NCHUNK * IDX2_CHUNK_COLS), dtype=np.int16)
    streams = [np.asarray(xs), np.asarray(path), np.asarray(xt)]
    for c in range(NCHUNK):
        col1 = c * IDX1_CHUNK_COLS
        for s in range(3):
            vals = streams[s][:, c * CL:(c + 1) * CL].T.reshape(-1).astype(np.int64)
            caps = STREAM_CAPS[s]
            bucket_of = vals // BUCKET
            pos = np.zeros(CHUNK_TOK, dtype=np.int64)
            off = 0
            for k, cap in enumerate(caps):
                members = np.nonzero(bucket_of == k)[0]
                cnt = len(members)
                if cnt > cap:
                    raise RuntimeError(f"bucket overflow: stream {s} bucket {k}: {cnt} > {cap}")
                loc = np.zeros(cap, dtype=np.int16)
                loc[:cnt] = (vals[members] - k * BUCKET).astype(np.int16)
                idx1[:, col1:col1 + cap // 16] = _wrap16(loc, cap // 16)
                col1 += cap // 16
                pos[members] = off + np.arange(cnt)
                off += cap
            col2 = c * IDX2_CHUNK_COLS + s * (CHUNK_TOK // 16)
            idx2[:, col2:col2 + CHUNK_TOK // 16] = _wrap16(
                pos.astype(np.int16), CHUNK_TOK // 16
            )
    return idx1, idx2


def prepare_in_maps(inputs):
    word_bf = np.ascontiguousarray(np.asarray(inputs["word_emb"], dtype=np.float32).astype(BF16))
    path_bf = np.ascontiguousarray(np.asarray(inputs["path_emb"], dtype=np.float32).astype(BF16))
    W = np.asarray(inputs["W_fc"], dtype=np.float32)          # [D, 3E]
    attn = np.asarray(inputs["attn"], dtype=np.float32)       # [D, 1]
    WT = W.T                                                  # [3E, D]
    wt_host = np.ascontiguousarray(
        WT.reshape(3, 128, D).transpose(1, 0, 2).astype(BF16)
    )                                                          # [128, 3, D]
    attn_rep = np.ascontiguousarray(
        np.broadcast_to(attn[:, 0][None, :], (128, D)).astype(BF16)
    )
    ident = np.eye(128, dtype=np.float32).astype(BF16)

    in_maps = []
    for core in range(NCORES):
        ext = core // 4
        b0 = (core % 4) * BC
        if ext == 0:
            xs = np.asarray(inputs["x_s1"])[b0:b0 + BC]
            pa = np.asarray(inputs["path1"])[b0:b0 + BC]
            xt = np.asarray(inputs["x_t1"])[b0:b0 + BC]
        else:
            xs = np.asarray(inputs["x_s2"])[b0:b0 + BC]
            pa = np.asarray(inputs["path2"])[b0:b0 + BC]
            xt = np.asarray(inputs["x_t2"])[b0:b0 + BC]
        idx1, idx2 = _prep_indices(xs, pa, xt)
        in_maps.append({
            "tab_w": word_bf,
            "tab_p": path_bf,
            "idx1": idx1,
            "idx2": idx2,
            "wt": wt_host,
            "attn_rep": attn_rep,
            "ident": ident,
        })
    return in_maps


def kernel(**inputs):
    nc = build_program()
    in_maps = prepare_in_maps(inputs)
    res = run_bass_kernel_spmd(nc, in_maps, core_ids=list(range(NCORES)))
    outs = [np.asarray(res.results[c]["out"], dtype=np.float32) for c in range(NCORES)]
    cv1 = np.concatenate(outs[:4], axis=0)
    cv2 = np.concatenate(outs[4:], axis=0)
    return (cv1, cv2)



# revision 22
# speedup vs baseline: 1.1395x; 1.1395x over previous
"""Code2VecEncoder Trainium2 kernel.

Strategy (8 NeuronCores):
  - cores 0-3: extract #1, batch rows [128c, 128(c+1))
  - cores 4-7: extract #2, same batch split
  Each core handles 128 batch rows x 200 paths of one extract end-to-end;
  no cross-core communication. Host concatenates per-core [128, 384]
  outputs into the two [512, 384] code vectors.

Per-core pipeline (l-major: token i = l*128 + b):
  1. phase-1 dma_gather: bucketed (int16-range) embedding-row gathers,
     HBM -> SBUF staging, bf16 rows (3 streams: word[xs], path, word[xt]).
  2. phase-2 dma_gather(transpose=True, SBUF source): permutation indices
     (token order) -> ctxT [f, tok] bf16. This both un-sorts the bucketed
     staging AND transposes rows into matmul ("f on partitions") layout.
  3. PE: pre[b, d] = sum_s ctxT_s.T @ WT_s  (bf16, fp32 PSUM accum)
  4. ACT: h = tanh(pre) -> bf16
  5. DVE: s[b] = sum_d h*attn  (fused tensor_tensor_reduce)
  6. ACT: e = exp(s)   (no max subtraction needed: |s| ~ 1)
  7. DVE: diag(e) = identity * e ; PE: cv_psum += diag(e) @ h  (200 accum)
  8. Z = sum_l e ; cv = cv_psum / Z ; DMA out.
"""

import numpy as np
import ml_dtypes

import concourse.bacc as bacc
import concourse.mybir as mybir
import concourse.tile as tile
from concourse import bass
from concourse.bass_utils import run_bass_kernel_spmd

BF16 = ml_dtypes.bfloat16

B, L = 512, 200
WORD_V, PATH_V = 100000, 200000
E, D = 128, 384
NCORES = 8
BC = 128                      # batch rows per core
BUCKET = 32768

CL = 100                      # l-values per staging chunk
NCHUNK = L // CL              # 2
CHUNK_TOK = CL * BC           # 12800 tokens per chunk per stream
SUB_TOK = 3200                # phase-2 subchunk (25 l)
NSUB = CHUNK_TOK // SUB_TOK   # 4
SUB_L = SUB_TOK // BC         # 25
NL_T = 3                      # l per PSUM-pre tile / tanh group

WORD_CAPS = [4864, 4864, 4864, 640]          # padded bucket slot caps
PATH_CAPS = [2816] * 6 + [640]
WORD_SLOTS = sum(WORD_CAPS)                   # 15232
PATH_SLOTS = sum(PATH_CAPS)                   # 17536
STREAM_CAPS = [WORD_CAPS, PATH_CAPS, WORD_CAPS]
STREAM_SLOTS = [WORD_SLOTS, PATH_SLOTS, WORD_SLOTS]
IDX1_CHUNK_COLS = sum(STREAM_SLOTS) // 16     # 3000
IDX2_CHUNK_COLS = 3 * (CHUNK_TOK // 16)       # 2400

_nc_cache = {}
_patch_stats = {}


def _l_groups():
    gs = []
    l0 = 0
    while l0 < SUB_L:
        g = min(NL_T, SUB_L - l0)
        gs.append((l0, g))
        l0 += g
    return gs


def _queue_pure_lane_patch():
    """Patch TileClockTick._assign_tick so Pool SWDGE DMAs land on
    queue-pure DMASW lanes.

    Tile's stock round-robin puts consecutive Pool DMAs on consecutive
    DMASW sem lanes. A lane's tick/wait semantics assume DMAs on it
    complete in order, which holds with a single SWDGE queue (one FIFO
    descriptor ring) but not across 4 queues. Mapping queue q to lanes
    {q, q+4} keeps every lane's DMAs on one queue, restoring in-order
    completion per lane.
    """
    from concourse import tile_sem_assignment as tsa
    from concourse.tile_scheduler import DMAInst, PROC_NAME_TO_IDX
    from concourse import bass_isa as bisa

    orig = tsa.TileClockTick._assign_tick
    _patch_stats.clear()

    def patched(self, inst):
        if (
            isinstance(inst, DMAInst)
            and not isinstance(inst, bisa.UserSyncedRemoteDMADescs)
            and inst.engine == mybir.EngineType.Pool
        ):
            _patch_stats["pool_dma"] = _patch_stats.get("pool_dma", 0) + 1
            _patch_stats.setdefault("queues", set()).add(getattr(inst, "queue_num", None))
            q = getattr(inst, "queue_num", 0) or 0
            toggles = getattr(self, "_q_lane_toggle", None)
            if toggles is None:
                toggles = self._q_lane_toggle = {}
            t = toggles.get(q, 0)
            toggles[q] = t ^ 1
            self.next_sw_dma_idx = (q + 4 * t) % self.swdge_sem_count
        return orig(self, inst)

    tsa.TileClockTick._assign_tick = patched
    return tsa, orig


def build_program():
    if "nc" in _nc_cache:
        return _nc_cache["nc"]
    nc = bacc.Bacc(
        "TRN2",
        target_bir_lowering=False,
        debug=False,
        enable_asserts=False,
        num_devices=NCORES,
        num_swdge_queues=4,
    )
    dt = mybir.dt
    tab_w = nc.dram_tensor("tab_w", [WORD_V, E], dt.bfloat16, kind="ExternalInput").ap()
    tab_p = nc.dram_tensor("tab_p", [PATH_V, E], dt.bfloat16, kind="ExternalInput").ap()
    idx1 = nc.dram_tensor("idx1", [128, NCHUNK * IDX1_CHUNK_COLS], dt.int16, kind="ExternalInput").ap()
    idx2 = nc.dram_tensor("idx2", [128, NCHUNK * IDX2_CHUNK_COLS], dt.int16, kind="ExternalInput").ap()
    wt = nc.dram_tensor("wt", [128, 3, D], dt.bfloat16, kind="ExternalInput").ap()
    attn_rep = nc.dram_tensor("attn_rep", [128, D], dt.bfloat16, kind="ExternalInput").ap()
    ident = nc.dram_tensor("ident", [128, 128], dt.bfloat16, kind="ExternalInput").ap()
    out = nc.dram_tensor("out", [BC, D], dt.float32, kind="ExternalOutput").ap()

    tabs = [tab_w, tab_p, tab_w]

    _tsa, _orig_assign = _queue_pure_lane_patch()
    try:
        _build_body(nc, tabs, tab_w, tab_p, idx1, idx2, wt, attn_rep, ident, out)
    finally:
        _tsa.TileClockTick._assign_tick = _orig_assign
    import sys as _sys
    print(f"lane patch stats: {_patch_stats}", file=_sys.stderr)

    nc.compile()
    _nc_cache["nc"] = nc
    return nc


def _build_body(nc, tabs, tab_w, tab_p, idx1, idx2, wt, attn_rep, ident, out):
    dt = mybir.dt
    with tile.TileContext(nc) as tc:
        with (
            tc.tile_pool(name="const", bufs=1) as constp,
            tc.tile_pool(name="stag", bufs=1) as stagp,
            tc.tile_pool(name="ctxT", bufs=2) as ctxp,
            tc.tile_pool(name="h", bufs=4) as hp,
            tc.tile_pool(name="small", bufs=3) as smallp,
            tc.tile_pool(name="pre", bufs=2, space="PSUM") as prep,
            tc.tile_pool(name="cvp", bufs=1, space="PSUM") as cvp,
        ):
            wt_sb = constp.tile([128, 3, D], dt.bfloat16)
            nc.sync.dma_start(out=wt_sb[:], in_=wt[:])
            attn_sb = constp.tile([128, D], dt.bfloat16)
            nc.sync.dma_start(out=attn_sb[:], in_=attn_rep[:])
            id_sb = constp.tile([128, 128], dt.bfloat16)
            nc.sync.dma_start(out=id_sb[:], in_=ident[:])
            idx1_sb = constp.tile([128, NCHUNK * IDX1_CHUNK_COLS], dt.int16)
            nc.sync.dma_start(out=idx1_sb[:], in_=idx1[:])
            idx2_sb = constp.tile([128, NCHUNK * IDX2_CHUNK_COLS], dt.int16)
            nc.sync.dma_start(out=idx2_sb[:], in_=idx2[:])

            s_all = constp.tile([128, L], dt.float32)
            e_all = constp.tile([128, L], dt.float32)
            junk = constp.tile([128, D], dt.bfloat16)
            zcol = constp.tile([128, 1], dt.float32)
            rz = constp.tile([128, 1], dt.float32)
            cv_sb = constp.tile([128, D], dt.float32)

            cv_ps = cvp.tile([128, 512], dt.float32)

            stream_tags = ["stg_s", "stg_p", "stg_t"]

            # Queue plan: the xbar transpose unit assembles 16-row tiles
            # statefully per DMA engine, so TRANSPOSE gathers from different
            # queues must never interleave -> all phase-2 gathers go on
            # queue 0. Phase-1 (plain row gathers, no xbar) spreads across
            # queues 1-3 by stream, overlapping with phase-2 descriptor
            # drain on queue 0's rings.

            for c in range(NCHUNK):
                stags = [
                    stagp.tile(
                        [128, STREAM_SLOTS[s] // 128, E], dt.bfloat16,
                        tag=stream_tags[s], name=f"{stream_tags[s]}_{c}",
                    )
                    for s in range(3)
                ]
                # ---- phase 1: bucketed HBM gathers into staging ----
                col = c * IDX1_CHUNK_COLS
                for s in range(3):
                    tab = tabs[s]
                    vocab = tab.shape[0]
                    blk0 = 0
                    for k, cap in enumerate(STREAM_CAPS[s]):
                        r0 = k * BUCKET
                        r1 = min(vocab, r0 + BUCKET)
                        nc.gpsimd.dma_gather(
                            out_ap=stags[s][:, blk0:blk0 + cap // 128, :],
                            in_ap=tab[r0:r1, :],
                            idxs_ap=idx1_sb[:, col:col + cap // 16],
                            num_idxs=cap,
                            num_idxs_reg=cap,
                            elem_size=E,
                            single_packet=False,
                            queue_num=1 + s,
                        )
                        col += cap // 16
                        blk0 += cap // 128

                # ---- phase 2 + compute, per 3200-token subchunk ----
                for sub in range(NSUB):
                    ctxT = ctxp.tile([128, 3, SUB_TOK], dt.bfloat16)
                    for s in range(3):
                        i2col = c * IDX2_CHUNK_COLS + s * (CHUNK_TOK // 16) + sub * (SUB_TOK // 16)
                        nc.gpsimd.dma_gather(
                            out_ap=ctxT[:, s:s + 1, :],
                            in_ap=stags[s][:],
                            idxs_ap=idx2_sb[:, i2col:i2col + SUB_TOK // 16],
                            num_idxs=SUB_TOK,
                            num_idxs_reg=SUB_TOK,
                            elem_size=E,
                            transpose=True,
                            sbuf_tokens_per_rank=128,
                            sbuf_free_dim_per_rank=E * 2,
                            sbuf_free_dim_pad_per_rank=0,
                            sbuf_byte_offset=0,
                            single_packet=False,
                            queue_num=0,
                        )

                    for (lg0, g) in _l_groups():
                        pre = prep.tile([128, NL_T, 512], dt.float32, tag="pre")
                        h = hp.tile([128, NL_T, D], dt.bfloat16, tag="h")
                        base_l = c * CL + sub * SUB_L + lg0
                        for j in range(g):
                            t0 = (lg0 + j) * BC
                            for s in range(3):
                                nc.tensor.matmul(
                                    pre[:, j, :D],
                                    lhsT=ctxT[:, s, t0:t0 + BC],
                                    rhs=wt_sb[:, s, :],
                                    start=(s == 0),
                                    stop=(s == 2),
                                    skip_group_check=True,
                                )
                        nc.scalar.activation(
                            h[:, :g, :], pre[:, :g, :D],
                            mybir.ActivationFunctionType.Tanh,
                        )
                        for j in range(g):
                            nc.vector.scalar_tensor_tensor(
                                out=junk[:],
                                in0=h[:, j, :],
                                scalar=1.0,
                                in1=attn_sb[:],
                                op0=mybir.AluOpType.bypass,
                                op1=mybir.AluOpType.mult,
                                accum_out=s_all[:, base_l + j:base_l + j + 1],
                            )
                        nc.scalar.activation(
                            e_all[:, base_l:base_l + g],
                            s_all[:, base_l:base_l + g],
                            mybir.ActivationFunctionType.Exp,
                        )
                        for j in range(g):
                            lglob = base_l + j
                            diag = smallp.tile([128, 128], dt.bfloat16, tag="diag")
                            nc.vector.tensor_scalar_mul(
                                diag[:], id_sb[:], e_all[:, lglob:lglob + 1]
                            )
                            nc.tensor.matmul(
                                cv_ps[:, :D],
                                lhsT=diag[:],
                                rhs=h[:, j, :],
                                start=(lglob == 0),
                                stop=(lglob == L - 1),
                                skip_group_check=True,
                            )

            nc.vector.tensor_reduce(
                out=zcol[:], in_=e_all[:], axis=mybir.AxisListType.X,
                op=mybir.AluOpType.add,
            )
            nc.vector.reciprocal(rz[:], zcol[:])
            nc.scalar.activation(
                cv_sb[:], cv_ps[:, :D],
                mybir.ActivationFunctionType.Copy,
                scale=rz[:, :1],
            )
            nc.sync.dma_start(out=out[:], in_=cv_sb[:])


def _wrap16(vals, ncols):
    """int16 values j -> partition j%16, col j//16; replicated to 128 parts."""
    m = np.zeros((16, ncols), dtype=np.int16)
    j = np.arange(len(vals))
    m[j % 16, j // 16] = vals
    return np.tile(m, (8, 1))


def _prep_indices(xs, path, xt):
    """Build idx1/idx2 arrays for one core.

    xs/path/xt: int arrays [128, 200] (this core's shard).
    Token order within chunk c: i = l_local*128 + b.
    """
    idx1 = np.zeros((128, NCHUNK * IDX1_CHUNK_COLS), dtype=np.int16)
    idx2 = np.zeros((128, NCHUNK * IDX2_CHUNK_COLS), dtype=np.int16)
    streams = [np.asarray(xs), np.asarray(path), np.asarray(xt)]
    for c in range(NCHUNK):
        col1 = c * IDX1_CHUNK_COLS
        for s in range(3):
            vals = streams[s][:, c * CL:(c + 1) * CL].T.reshape(-1).astype(np.int64)
            caps = STREAM_CAPS[s]
            bucket_of = vals // BUCKET
            pos = np.zeros(CHUNK_TOK, dtype=np.int64)
            off = 0
            for k, cap in enumerate(caps):
                members = np.nonzero(bucket_of == k)[0]
                cnt = len(members)
                if cnt > cap:
                    raise RuntimeError(f"bucket overflow: stream {s} bucket {k}: {cnt} > {cap}")
                loc = np.zeros(cap, dtype=np.int16)
                loc[:cnt] = (vals[members] - k * BUCKET).astype(np.int16)
                idx1[:, col1:col1 + cap // 16] = _wrap16(loc, cap // 16)
                col1 += cap // 16
                pos[members] = off + np.arange(cnt)
                off += cap
            col2 = c * IDX2_CHUNK_COLS + s * (CHUNK_TOK // 16)
            idx2[:, col2:col2 + CHUNK_TOK // 16] = _wrap16(
                pos.astype(np.int16), CHUNK_TOK // 16
            )
    return idx1, idx2


def prepare_in_maps(inputs):
    word_bf = np.ascontiguousarray(np.asarray(inputs["word_emb"], dtype=np.float32).astype(BF16))
    path_bf = np.ascontiguousarray(np.asarray(inputs["path_emb"], dtype=np.float32).astype(BF16))
    W = np.asarray(inputs["W_fc"], dtype=np.float32)          # [D, 3E]
    attn = np.asarray(inputs["attn"], dtype=np.float32)       # [D, 1]
    WT = W.T                                                  # [3E, D]
    wt_host = np.ascontiguousarray(
        WT.reshape(3, 128, D).transpose(1, 0, 2).astype(BF16)
    )                                                          # [128, 3, D]
    attn_rep = np.ascontiguousarray(
        np.broadcast_to(attn[:, 0][None, :], (128, D)).astype(BF16)
    )
    ident = np.eye(128, dtype=np.float32).astype(BF16)

    in_maps = []
    for core in range(NCORES):
        ext = core // 4
        b0 = (core % 4) * BC
        if ext == 0:
            xs = np.asarray(inputs["x_s1"])[b0:b0 + BC]
            pa = np.asarray(inputs["path1"])[b0:b0 + BC]
            xt = np.asarray(inputs["x_t1"])[b0:b0 + BC]
        else:
            xs = np.asarray(inputs["x_s2"])[b0:b0 + BC]
            pa = np.asarray(inputs["path2"])[b0:b0 + BC]
            xt = np.asarray(inputs["x_t2"])[b0:b0 + BC]
        idx1, idx2 = _prep_indices(xs, pa, xt)
        in_maps.append({
            "tab_w": word_bf,
            "tab_p": path_bf,
            "idx1": idx1,
            "idx2": idx2,
            "wt": wt_host,
            "attn_rep": attn_rep,
            "ident": ident,
        })
    return in_maps


def kernel(**inputs):
    nc = build_program()
    in_maps = prepare_in_maps(inputs)
    res = run_bass_kernel_spmd(nc, in_maps, core_ids=list(range(NCORES)))
    outs = [np.asarray(res.results[c]["out"], dtype=np.float32) for c in range(NCORES)]
    cv1 = np.concatenate(outs[:4], axis=0)
    cv2 = np.concatenate(outs[4:], axis=0)
    return (cv1, cv2)

